# revision 40
# baseline (speedup 1.0000x reference)
"""Multi-head causal attention (B=4, S=2048, D=1024, H=16) on 8 TRN2 NeuronCores.

Sharding: batch x head-group. Core c handles batch c//2 and heads
8*(c%2) .. 8*(c%2)+8 (tensor parallel over heads). Each core computes its
8 heads' attention plus partial output projections; the host sums the four
partials per batch and adds b_out.

Device pipeline (per core) — single woven instruction stream:
  - head pairs processed sequentially (oT needs only 2 PSUM banks), which
    frees 2 PSUM banks for a projection-chain pool that stays live through
    the attention phase: qk/v/out-projection chains are emitted as PE
    filler between attention events, paced ~RATE matmuls per event, so the
    PE never drains while ScalarE runs exp and ScalarE never waits on
    scores.
  - scores via row-packed (tile_position) fp16 matmuls in S^T [k, q]
    layout, exp on ScalarE straight out of PSUM, fp16 P with
    multiplicative mask tiles for diagonal blocks; fully-masked blocks
    skipped, fully-masked leading columns excluded from score matmul, exp,
    AND attn@V (N-trimmed).
  - attn@V with lhsT = [v_h | ones] (M=65): row 64 accumulates softmax
    denominators; normalization = reciprocal_approx_fast + GpSimd
    partition broadcast + multiply.
  - out-projection split into two f16 partials (pairs 0-1 / pairs 2-3) so
    d-chains weave in as soon as their pair-group's rows normalize; host
    sums partials.
"""
import collections

import numpy as np

import concourse.bass as bass
import concourse.tile as tile
from concourse import bacc, mybir
from concourse import bass_utils

B, S, D, H, HD = 4, 2048, 1024, 16, 64
NCORES = 8
HPC = H // 2          # heads per core (8)
NPAIR = HPC // 2      # head pairs per core (4)
DC = HPC * HD         # attn dims per core (512)
QT = 512              # q tile (free dim of S^T)
KT = 128              # k tile (partition dim of S^T)
NQT = S // QT         # 4
NKT = S // KT         # 16
NTT = S // 128        # 16 token tiles
NCH = D // 128        # 8 d_model chunks
SCALE = HD ** -0.5
LAG = 4               # events of exp lookahead before attn@V
RATE = 2.2            # filler matmul-slots released per attention event


def _wqk_col(sl):
    """Column offset of qk slice sl in the [q01|k01|q23|k23] wqk layout."""
    p = sl if sl < NPAIR else sl - NPAIR
    return (p // 2) * 512 + (256 if sl >= NPAIR else 0) + (p % 2) * 128

F32 = mybir.dt.float32
F16 = mybir.dt.float16

_cache = {}


def _classify_mask(mask):
    """Per (kt, qt) block: 0=skip (all masked), 1=full (none masked), 2=partial."""
    mask = np.asarray(mask).astype(bool)
    classes = np.zeros((NKT, NQT), np.int8)
    patterns = []
    pat_idx = {}
    pat_key = {}
    bounds = {}
    for qt in range(NQT):
        mb = mask[qt * QT:(qt + 1) * QT, :]          # [512, S] (q, k)
        for kt in range(NKT):
            blk = mb[:, kt * KT:(kt + 1) * KT]       # [512, 128] (q, k)
            if blk.all():
                classes[kt, qt] = 0
            elif not blk.any():
                classes[kt, qt] = 1
            else:
                classes[kt, qt] = 2
                tilev = (~blk).T.astype(np.float32)  # [128, 512] (k, q), 1=keep
                col_masked = tilev.min(axis=0) == 0.0
                col_dead = tilev.max(axis=0) == 0.0
                zlo = 0
                while zlo < QT and col_dead[zlo]:
                    zlo += 1
                nz = np.nonzero(col_masked[zlo:])[0]
                if len(nz):
                    mlo, mhi = zlo + int(nz[0]), zlo + int(nz[-1]) + 1
                else:
                    mlo, mhi = zlo, zlo
                key = tilev.tobytes()
                if key not in pat_key:
                    pat_key[key] = len(patterns)
                    patterns.append(tilev)
                pat_idx[(kt, qt)] = pat_key[key]
                bounds[(kt, qt)] = (zlo, mlo, mhi)
    if not patterns:
        patterns.append(np.ones((KT, QT), np.float32))
    return classes, np.stack(patterns), pat_idx, bounds


def _build(classes, pat_idx, bounds, n_pat):
    nc = bacc.Bacc("TRN2", target_bir_lowering=False, debug=False,
                   num_devices=NCORES)

    xT_d = nc.dram_tensor("xT", [D, S], F16, kind="ExternalInput")
    wqk_d = nc.dram_tensor("wqk", [D, 2 * DC], F16, kind="ExternalInput")
    wv_d = nc.dram_tensor("wv", [D, DC], F16, kind="ExternalInput")
    wo_d = nc.dram_tensor("wo", [DC, D], F16, kind="ExternalInput")
    bqk_d = nc.dram_tensor("bqk", [128, 2 * NPAIR], F32, kind="ExternalInput")
    bv_d = nc.dram_tensor("bv", [1, DC], F32, kind="ExternalInput")
    mpat_d = nc.dram_tensor("mpat", [n_pat, KT, QT], F16, kind="ExternalInput")
    outA_d = nc.dram_tensor("outA", [S, D], F16, kind="ExternalOutput")
    outB_d = nc.dram_tensor("outB", [S, D], F16, kind="ExternalOutput")

    wqk_c = wqk_d.ap().rearrange("(c p) n -> c p n", p=128)
    wv_c = wv_d.ap().rearrange("(c p) n -> c p n", p=128)
    wo_c = wo_d.ap().rearrange("(c p) n -> c p n", p=128)
    xT_c = xT_d.ap().rearrange("(c p) n -> c p n", p=128)

    with tile.TileContext(nc) as tc:
        with (
            tc.tile_pool(name="persist", bufs=1) as persist,
            tc.tile_pool(name="bigpool", bufs=1) as bigpool,
            tc.tile_pool(name="ppool", bufs=8) as ppool,
            tc.tile_pool(name="spool", bufs=4) as spool,
            tc.tile_pool(name="dcopy", bufs=8) as dcopy,
            tc.tile_pool(name="psS", bufs=2, space="PSUM") as psS,
            tc.tile_pool(name="psO", bufs=1, space="PSUM") as psO,
            tc.tile_pool(name="psA", bufs=2, space="PSUM") as psA,
        ):
            # ---- persistent tiles + input DMAs (ramp-ordered) ---------
            qkT = [bigpool.tile([128, S], F16, name=f"qkT{p}", tag="qk",
                                bufs=8) for p in range(2 * NPAIR)]
            vext = [persist.tile([128, HPC, HD + 1], F16, name=f"vx{t}",
                                 tag=f"vx{t}") for t in range(NTT)]
            mpat = [persist.tile([KT, QT], F16, name=f"mp{i}", tag=f"mp{i}")
                    for i in range(n_pat)]
            outTn = [bigpool.tile([128, S], F16, name=f"oTn{p}", tag="big",
                                  bufs=4) for p in range(NPAIR)]
            bqk_sb = persist.tile([128, 2 * NPAIR], F32)
            bv_bc = persist.tile([128, DC], F32)
            xT = [persist.tile([128, S], F16, name=f"x{ch}", tag=f"x{ch}")
                  for ch in range(NCH)]
            wqk_sb = [persist.tile([128, 2 * DC], F16, name=f"wqk{ch}",
                                   tag=f"wqk{ch}") for ch in range(NCH)]
            wv_sb = [persist.tile([128, DC], F16, name=f"wv{ch}",
                                  tag=f"wv{ch}") for ch in range(NCH)]
            wo_sb = [persist.tile([128, D], F16, name=f"wo{p}", tag=f"wo{p}")
                     for p in range(NPAIR)]

            # Critical ramp DMAs only; the bulk is emitted later behind
            # artificial WAR guards (the 16 HW DMA queues fair-share
            # bandwidth, so issuing everything at t=0 starves the ramp).
            nc.sync.dma_start(bqk_sb, bqk_d.ap())
            nc.sync.dma_start(
                bv_bc,
                bass.AP(tensor=bv_d, offset=0, ap=[[0, 128], [1, DC]]))
            for i in range(n_pat):
                nc.sync.dma_start(mpat[i], mpat_d.ap()[i])
            # interleave per-chunk: the first chain's MMs consume ch-chunks
            # in order, so they can start once chunk 0 of each operand lands
            for ch in range(NCH):
                nc.sync.dma_start(wqk_sb[ch][:, 0:512], wqk_c[ch][:, 0:512])
                nc.sync.dma_start(xT[ch][:, 0:QT], xT_c[ch][:, 0:QT])
            for ch in range(NCH):
                nc.sync.dma_start(wv_sb[ch], wv_c[ch])

            guard = persist.tile([1, 64], F16, name="guard")
            guard_n = [0]

            def deferred_dma(trig, dst_probe, dst, src):
                """Emit DMA gated behind trig via a WAR-creating dummy read."""
                g = guard_n[0]
                guard_n[0] += 1
                nc.vector.tensor_add(guard[0:1, g:g + 1], trig, dst_probe)
                nc.sync.dma_start(dst, src)

            def emit_tier(tier):
                # tier 2: x chunks tt1-2 once the ramp's first chain runs;
                # tier 3: x tt3 + wqk q23/k23 half; tier 4: w_out.
                if tier == 2:
                    trig = qkT[0][0:1, 0:1]
                    for tt in (1, 2):
                        for ch in range(NCH):
                            sl = slice(tt * QT, (tt + 1) * QT)
                            deferred_dma(trig, xT[ch][0:1, tt * QT:tt * QT + 1],
                                         xT[ch][:, sl], xT_c[ch][:, sl])
                elif tier == 3:
                    trig = qkT[0][0:1, QT:QT + 1]
                    for ch in range(NCH):
                        sl = slice(3 * QT, 4 * QT)
                        deferred_dma(trig, xT[ch][0:1, 3 * QT:3 * QT + 1],
                                     xT[ch][:, sl], xT_c[ch][:, sl])
                    for ch in range(NCH):
                        deferred_dma(trig, wqk_sb[ch][0:1, 512:513],
                                     wqk_sb[ch][:, 512:1024],
                                     wqk_c[ch][:, 512:1024])
                else:
                    trig = qkT[1][0:1, 0:1]
                    for p in range(NPAIR):
                        deferred_dma(trig, wo_sb[p][0:1, 0:1],
                                     wo_sb[p], wo_c[p])

            # ---- chain emitters ---------------------------------------
            def emit_qk_chain(p, tt):
                """qkT[p][:, tt*QT:...] = (w_qk[:, p-slice].T @ x.T) + bias."""
                ps = psA.tile([128, QT], F32, tag="pa", name=f"psqk{p}_{tt}")
                c0 = _wqk_col(p)
                for ch in range(NCH):
                    nc.tensor.matmul(
                        ps, wqk_sb[ch][:, c0:c0 + 128],
                        xT[ch][:, tt * QT:(tt + 1) * QT],
                        start=(ch == 0), stop=(ch == NCH - 1))
                nc.vector.tensor_scalar_add(
                    qkT[p][:, tt * QT:(tt + 1) * QT], ps, bqk_sb[:, p:p + 1])
                if (p, tt) == (0, 1):
                    emit_tier(3)
                elif (p, tt) == (1, 0):
                    emit_tier(4)

            def emit_v_chain(tt):
                """vext[tt] <- x[tt-tokens] @ w_v + bias, plus ones column."""
                ps = psA.tile([128, DC], F32, tag="pa", name=f"psv{tt}")
                for ch in range(NCH):
                    nc.tensor.matmul(
                        ps, xT[ch][:, tt * 128:(tt + 1) * 128], wv_sb[ch],
                        start=(ch == 0), stop=(ch == NCH - 1))
                src3 = ps.rearrange("p (h d) -> p h d", h=HPC)
                bv3 = bv_bc.rearrange("p (h d) -> p h d", h=HPC)
                nc.vector.tensor_add(vext[tt][:, :, 0:HD], src3, bv3)
                nc.vector.memset(vext[tt][:, :, HD:HD + 1], 1.0)

            def emit_d_chain(grp, tt, nt):
                """Partial out-projection for pair group grp (pairs 2g,2g+1)."""
                pairs = [2 * grp, 2 * grp + 1]
                dst = outA_d if grp == 0 else outB_d
                pso = psA.tile([128, QT], F32, name=f"pso{grp}_{tt}_{nt}",
                               tag="pa")
                for i, p in enumerate(pairs):
                    nc.tensor.matmul(
                        pso, outTn[p][:, tt * 128:(tt + 1) * 128],
                        wo_sb[p][:, nt * QT:(nt + 1) * QT],
                        start=(i == 0), stop=(i == len(pairs) - 1))
                ot = dcopy.tile([128, QT], F16, tag="oc")
                nc.vector.tensor_copy(ot, pso)
                nc.sync.dma_start(
                    dst.ap()[tt * 128:(tt + 1) * 128,
                             nt * QT:(nt + 1) * QT], ot)

            # ---- filler machinery -------------------------------------
            emitted = set()
            filler_q = collections.deque()
            state = {"quota": 0.0}

            def emit_unit(key):
                if key in emitted:
                    return
                emitted.add(key)
                if key[0] == "qk":
                    emit_qk_chain(key[1], key[2])
                elif key[0] == "v":
                    emit_v_chain(key[1])
                else:
                    emit_d_chain(key[1], key[2], key[3])

            def pop_fillers():
                while filler_q:
                    cost, key = filler_q[0]
                    if key in emitted:
                        filler_q.popleft()
                        continue
                    if state["quota"] < cost:
                        break
                    filler_q.popleft()
                    state["quota"] -= cost
                    emit_unit(key)

            # static queue: p0's remaining qk/v deps, then later pairs' qk
            for tt in range(1, NQT):
                filler_q.append((8, ("qk", 0, tt)))
                filler_q.append((8, ("qk", NPAIR, tt)))
                for kt in range(4 * tt, 4 * tt + 4):
                    filler_q.append((8, ("v", kt)))
            for p in range(1, NPAIR):
                for tt in range(NQT):
                    filler_q.append((8, ("qk", p, tt)))
                    filler_q.append((8, ("qk", NPAIR + p, tt)))

            # ---- attention emitters -----------------------------------
            oT_live = {}
            pAB_live = {}

            def emit_sexp(ev):
                p, qt, kt, first, last = ev
                qTp, kTp = qkT[p], qkT[NPAIR + p]
                if first:
                    oT_live[(p, qt)] = [
                        psO.tile([HD + 1, QT], F32,
                                 name=f"o{p}_{qt}_{h}", tag=f"o_{h}")
                        for h in range(2)]
                zlo, mlo, mhi = (0, 0, 0) if classes[kt, qt] == 1 \
                    else bounds[(kt, qt)]
                sAB = psS.tile([128, 2, QT], F32, tag="sAB",
                               name=f"s{p}_{qt}_{kt}")
                for h in range(2):
                    nc.tensor.matmul(
                        sAB[:, h, zlo:QT],
                        kTp[64 * h:64 * h + 64, kt * KT:(kt + 1) * KT],
                        qTp[64 * h:64 * h + 64, qt * QT + zlo:(qt + 1) * QT],
                        tile_position=(64 * h, 0))
                pAB = ppool.tile([128, 2, QT], F16, tag="pAB",
                                 name=f"p{p}_{qt}_{kt}")
                nc.scalar.activation(
                    pAB[:, :, zlo:QT], sAB[:, :, zlo:QT],
                    mybir.ActivationFunctionType.Exp, scale=SCALE)
                if mhi > mlo:
                    pap = mpat[pat_idx[(kt, qt)]][:, mlo:mhi]
                    bap = bass.AP(tensor=pap.tensor, offset=pap.offset,
                                  ap=[pap.ap[0], [0, 2]] + pap.ap[1:])
                    nc.vector.tensor_mul(
                        pAB[:, :, mlo:mhi], pAB[:, :, mlo:mhi], bap)
                if first and zlo:
                    # general-mask guard: first event must initialize the
                    # full oT width, so zero the dead columns and run av
                    # untrimmed (never hit for a causal mask: zlo==0).
                    nc.vector.memset(pAB[:, :, 0:zlo], 0.0)
                    zlo = 0
                pAB_live[(p, qt, kt)] = (pAB, zlo)

            def emit_av(ev):
                p, qt, kt, first, last = ev
                oT = oT_live[(p, qt)]
                pAB, zlo = pAB_live.pop((p, qt, kt))
                for h in range(2):
                    nc.tensor.matmul(
                        oT[h][:, zlo:QT], vext[kt][:, 2 * p + h, :],
                        pAB[:, h, zlo:QT], start=first, stop=last)
                if last:
                    for h in range(2):
                        den = spool.tile([1, QT], F32, tag="den",
                                         name=f"d{p}_{qt}_{h}")
                        nc.vector.tensor_copy(den, oT[h][HD:HD + 1, :])
                        rec = spool.tile([1, QT], F32, tag="rec",
                                         name=f"r{p}_{qt}_{h}")
                        nc.vector.reciprocal_approx_fast(out=rec, in_=den)
                        bc = spool.tile([HD, QT], F32, tag="bc",
                                        name=f"b{p}_{qt}_{h}")
                        nc.gpsimd.partition_broadcast(bc, rec[0:1, :])
                        nc.vector.tensor_mul(
                            outTn[p][64 * h:64 * h + 64,
                                     qt * QT:(qt + 1) * QT],
                            oT[h][0:HD, :], bc)
                    del oT_live[(p, qt)]
                    if p in (1, 3):
                        # pair-group rows for this qt are now final on both
                        # pairs; out-projection chains become emittable
                        for tt in range(4 * qt, 4 * qt + 4):
                            for nt in range(2):
                                filler_q.append((2, ("d", p // 2, tt, nt)))

            # ---- ramp: first qk chains, then v ------------------------
            with nc.named_scope("ramp"):
                emit_unit(("qk", 0, 0))
                emit_unit(("qk", NPAIR, 0))
                emit_tier(2)
                for kt in range(4):
                    emit_unit(("v", kt))

            # ---- main woven stream ------------------------------------
            all_events = []
            for p in range(NPAIR):
                for qt in range(NQT):
                    kts = [kt for kt in range(NKT) if classes[kt, qt] != 0]
                    for i, kt in enumerate(kts):
                        all_events.append(
                            (p, qt, kt, i == 0, i == len(kts) - 1))

            with nc.named_scope("attn"):
                for idx, ev in enumerate(all_events):
                    p, qt, kt, first, last = ev
                    emit_unit(("qk", p, qt))
                    emit_unit(("qk", NPAIR + p, kt // 4))
                    emit_unit(("v", kt))
                    emit_sexp(ev)
                    state["quota"] += RATE
                    j = idx - LAG
                    if j >= 0:
                        emit_av(all_events[j])
                    pop_fillers()
                for j in range(max(0, len(all_events) - LAG),
                               len(all_events)):
                    emit_av(all_events[j])

            with nc.named_scope("tail"):
                state["quota"] = float("inf")
                pop_fillers()

    nc.compile()
    return nc


def _prepare_inputs(x, mask, w_qkv, b_qkv, w_out):
    classes, patterns, pat_idx, bounds = _classify_mask(np.asarray(mask))
    in_maps = []
    for c in range(NCORES):
        b, g = c // 2, c % 2
        h0 = g * HPC
        xT = np.ascontiguousarray(x[b].T.astype(np.float16))
        wq = w_qkv[:, h0 * HD:h0 * HD + DC]
        wk = w_qkv[:, D + h0 * HD:D + h0 * HD + DC]
        wv = w_qkv[:, 2 * D + h0 * HD:2 * D + h0 * HD + DC]
        bq = b_qkv[h0 * HD:h0 * HD + DC]
        bk = b_qkv[D + h0 * HD:D + h0 * HD + DC]
        bv = b_qkv[2 * D + h0 * HD:2 * D + h0 * HD + DC]
        wo = w_out[h0 * HD:h0 * HD + DC, :]
        in_maps.append({
            "xT": xT,
            # column layout [q01 | k01 | q23 | k23] so the first half is
            # one contiguous ramp-critical DMA (see _wqk_col)
            "wqk": np.ascontiguousarray(
                np.concatenate([wq[:, 0:256], wk[:, 0:256],
                                wq[:, 256:512], wk[:, 256:512]],
                               axis=1).astype(np.float16)),
            "wv": np.ascontiguousarray(wv.astype(np.float16)),
            "wo": np.ascontiguousarray(wo.astype(np.float16)),
            "bqk": np.ascontiguousarray(
                np.concatenate([bq, bk]).reshape(2 * NPAIR, 128).T
                .astype(np.float32)),
            "bv": np.ascontiguousarray(bv[None, :].astype(np.float32)),
            "mpat": patterns.astype(np.float16),
        })
    return classes, patterns, pat_idx, bounds, in_maps


def run(x, mask, w_qkv, b_qkv, w_out, b_out, trace=False):
    classes, patterns, pat_idx, bounds, in_maps = _prepare_inputs(
        x, mask, w_qkv, b_qkv, w_out)
    key = (classes.tobytes(), patterns.tobytes())
    if key not in _cache:
        _cache[key] = _build(classes, pat_idx, bounds, patterns.shape[0])
    nc = _cache[key]
    res = bass_utils.run_bass_kernel_spmd(
        nc, in_maps, core_ids=list(range(NCORES)), trace=trace)
    out = np.empty((B, S, D), np.float32)
    bo = np.asarray(b_out, np.float32)
    for b in range(B):
        acc = np.zeros((S, D), np.float32)
        for c in (2 * b, 2 * b + 1):
            acc += res.results[c]["outA"].astype(np.float32)
            acc += res.results[c]["outB"].astype(np.float32)
        out[b] = acc + bo
    return out, res


def kernel(x, mask, w_qkv, b_qkv, w_out, b_out):
    out, _ = run(x, mask, w_qkv, b_qkv, w_out, b_out, trace=False)
    return out


# revision 42
# speedup vs baseline: 1.0171x; 1.0171x over previous
"""Multi-head causal attention (B=4, S=2048, D=1024, H=16) on 8 TRN2 NeuronCores.

Sharding: batch x head-group. Core c handles batch c//2 and heads
8*(c%2) .. 8*(c%2)+8 (tensor parallel over heads). Each core computes its
8 heads' attention plus partial output projections; the host sums the four
partials per batch and adds b_out.

Device pipeline (per core) — single woven instruction stream:
  - head pairs processed sequentially (oT needs only 2 PSUM banks), which
    frees 2 PSUM banks for a projection-chain pool that stays live through
    the attention phase: qk/v/out-projection chains are emitted as PE
    filler between attention events, paced ~RATE matmuls per event, so the
    PE never drains while ScalarE runs exp and ScalarE never waits on
    scores.
  - scores via row-packed (tile_position) fp16 matmuls in S^T [k, q]
    layout, exp on ScalarE straight out of PSUM, fp16 P with
    multiplicative mask tiles for diagonal blocks; fully-masked blocks
    skipped, fully-masked leading columns excluded from score matmul, exp,
    AND attn@V (N-trimmed).
  - attn@V with lhsT = [v_h | ones] (M=65): row 64 accumulates softmax
    denominators; normalization = reciprocal_approx_fast + GpSimd
    partition broadcast + multiply.
  - out-projection split into two f16 partials (pairs 0-1 / pairs 2-3) so
    d-chains weave in as soon as their pair-group's rows normalize; host
    sums partials.
"""
import collections

import numpy as np

import concourse.bass as bass
import concourse.tile as tile
from concourse import bacc, mybir
from concourse import bass_utils

B, S, D, H, HD = 4, 2048, 1024, 16, 64
NCORES = 8
HPC = H // 2          # heads per core (8)
NPAIR = HPC // 2      # head pairs per core (4)
DC = HPC * HD         # attn dims per core (512)
QT = 512              # q tile (free dim of S^T)
KT = 128              # k tile (partition dim of S^T)
NQT = S // QT         # 4
NKT = S // KT         # 16
NTT = S // 128        # 16 token tiles
NCH = D // 128        # 8 d_model chunks
SCALE = HD ** -0.5
LAG = 5               # events of exp lookahead before attn@V
RATE = 2.3            # filler matmul-slots released per attention event


def _wqk_col(sl):
    """Column offset of qk slice sl in the [q01|k01|q23|k23] wqk layout."""
    p = sl if sl < NPAIR else sl - NPAIR
    return (p // 2) * 512 + (256 if sl >= NPAIR else 0) + (p % 2) * 128

F32 = mybir.dt.float32
F16 = mybir.dt.float16

_cache = {}


def _classify_mask(mask):
    """Per (kt, qt) block: 0=skip (all masked), 1=full (none masked), 2=partial."""
    mask = np.asarray(mask).astype(bool)
    classes = np.zeros((NKT, NQT), np.int8)
    patterns = []
    pat_idx = {}
    pat_key = {}
    bounds = {}
    for qt in range(NQT):
        mb = mask[qt * QT:(qt + 1) * QT, :]          # [512, S] (q, k)
        for kt in range(NKT):
            blk = mb[:, kt * KT:(kt + 1) * KT]       # [512, 128] (q, k)
            if blk.all():
                classes[kt, qt] = 0
            elif not blk.any():
                classes[kt, qt] = 1
            else:
                classes[kt, qt] = 2
                tilev = (~blk).T.astype(np.float32)  # [128, 512] (k, q), 1=keep
                col_masked = tilev.min(axis=0) == 0.0
                col_dead = tilev.max(axis=0) == 0.0
                zlo = 0
                while zlo < QT and col_dead[zlo]:
                    zlo += 1
                nz = np.nonzero(col_masked[zlo:])[0]
                if len(nz):
                    mlo, mhi = zlo + int(nz[0]), zlo + int(nz[-1]) + 1
                else:
                    mlo, mhi = zlo, zlo
                key = tilev.tobytes()
                if key not in pat_key:
                    pat_key[key] = len(patterns)
                    patterns.append(tilev)
                pat_idx[(kt, qt)] = pat_key[key]
                bounds[(kt, qt)] = (zlo, mlo, mhi)
    if not patterns:
        patterns.append(np.ones((KT, QT), np.float32))
    return classes, np.stack(patterns), pat_idx, bounds


def _build(classes, pat_idx, bounds, n_pat):
    nc = bacc.Bacc("TRN2", target_bir_lowering=False, debug=False,
                   num_devices=NCORES)

    xT_d = nc.dram_tensor("xT", [D, S], F16, kind="ExternalInput")
    wqk_d = nc.dram_tensor("wqk", [D, 2 * DC], F16, kind="ExternalInput")
    wv_d = nc.dram_tensor("wv", [D, DC], F16, kind="ExternalInput")
    wo_d = nc.dram_tensor("wo", [DC, D], F16, kind="ExternalInput")
    bqk_d = nc.dram_tensor("bqk", [128, 2 * NPAIR], F32, kind="ExternalInput")
    bv_d = nc.dram_tensor("bv", [1, DC], F32, kind="ExternalInput")
    mpat_d = nc.dram_tensor("mpat", [n_pat, KT, QT], F16, kind="ExternalInput")
    outA_d = nc.dram_tensor("outA", [S, D], F16, kind="ExternalOutput")
    outB_d = nc.dram_tensor("outB", [S, D], F16, kind="ExternalOutput")

    wqk_c = wqk_d.ap().rearrange("(c p) n -> c p n", p=128)
    wv_c = wv_d.ap().rearrange("(c p) n -> c p n", p=128)
    wo_c = wo_d.ap().rearrange("(c p) n -> c p n", p=128)
    xT_c = xT_d.ap().rearrange("(c p) n -> c p n", p=128)

    with tile.TileContext(nc) as tc:
        with (
            tc.tile_pool(name="persist", bufs=1) as persist,
            tc.tile_pool(name="bigpool", bufs=1) as bigpool,
            tc.tile_pool(name="ppool", bufs=8) as ppool,
            tc.tile_pool(name="spool", bufs=4) as spool,
            tc.tile_pool(name="dcopy", bufs=8) as dcopy,
            tc.tile_pool(name="psS", bufs=2, space="PSUM") as psS,
            tc.tile_pool(name="psO", bufs=1, space="PSUM") as psO,
            tc.tile_pool(name="psA", bufs=2, space="PSUM") as psA,
        ):
            # ---- persistent tiles + input DMAs (ramp-ordered) ---------
            qkT = [bigpool.tile([128, S], F16, name=f"qkT{p}", tag="qk",
                                bufs=8) for p in range(2 * NPAIR)]
            vext = [persist.tile([128, HPC, HD + 1], F16, name=f"vx{t}",
                                 tag=f"vx{t}") for t in range(NTT)]
            mpat = [persist.tile([KT, QT], F16, name=f"mp{i}", tag=f"mp{i}")
                    for i in range(n_pat)]
            outTn = [bigpool.tile([128, S], F16, name=f"oTn{p}", tag="big",
                                  bufs=4) for p in range(NPAIR)]
            bqk_sb = persist.tile([128, 2 * NPAIR], F32)
            bv_bc = persist.tile([128, DC], F32)
            xT = [persist.tile([128, S], F16, name=f"x{ch}", tag=f"x{ch}")
                  for ch in range(NCH)]
            wqk_sb = [persist.tile([128, 2 * DC], F16, name=f"wqk{ch}",
                                   tag=f"wqk{ch}") for ch in range(NCH)]
            wv_sb = [persist.tile([128, DC], F16, name=f"wv{ch}",
                                  tag=f"wv{ch}") for ch in range(NCH)]
            wo_sb = [persist.tile([128, D], F16, name=f"wo{p}", tag=f"wo{p}")
                     for p in range(NPAIR)]

            # Critical ramp DMAs only; the bulk is emitted later behind
            # artificial WAR guards (the 16 HW DMA queues fair-share
            # bandwidth, so issuing everything at t=0 starves the ramp).
            nc.sync.dma_start(bqk_sb, bqk_d.ap())
            nc.sync.dma_start(
                bv_bc,
                bass.AP(tensor=bv_d, offset=0, ap=[[0, 128], [1, DC]]))
            for i in range(n_pat):
                nc.sync.dma_start(mpat[i], mpat_d.ap()[i])
            for ch in range(NCH):
                nc.sync.dma_start(wqk_sb[ch][:, 0:512], wqk_c[ch][:, 0:512])
            for ch in range(NCH):
                nc.sync.dma_start(xT[ch][:, 0:QT], xT_c[ch][:, 0:QT])
            for ch in range(NCH):
                nc.sync.dma_start(wv_sb[ch], wv_c[ch])

            guard = persist.tile([1, 64], F16, name="guard")
            guard_n = [0]

            def deferred_dma(trig, dst_probe, dst, src):
                """Emit DMA gated behind trig via a WAR-creating dummy read."""
                g = guard_n[0]
                guard_n[0] += 1
                nc.vector.tensor_add(guard[0:1, g:g + 1], trig, dst_probe)
                nc.sync.dma_start(dst, src)

            def emit_tier(tier):
                # tier 2: x chunks tt1-2 once the ramp's first chain runs;
                # tier 3: x tt3 + wqk q23/k23 half; tier 4: w_out.
                if tier == 2:
                    trig = qkT[0][0:1, 0:1]
                    for tt in (1, 2):
                        for ch in range(NCH):
                            sl = slice(tt * QT, (tt + 1) * QT)
                            deferred_dma(trig, xT[ch][0:1, tt * QT:tt * QT + 1],
                                         xT[ch][:, sl], xT_c[ch][:, sl])
                elif tier == 3:
                    trig = qkT[0][0:1, QT:QT + 1]
                    for ch in range(NCH):
                        sl = slice(3 * QT, 4 * QT)
                        deferred_dma(trig, xT[ch][0:1, 3 * QT:3 * QT + 1],
                                     xT[ch][:, sl], xT_c[ch][:, sl])
                    for ch in range(NCH):
                        deferred_dma(trig, wqk_sb[ch][0:1, 512:513],
                                     wqk_sb[ch][:, 512:1024],
                                     wqk_c[ch][:, 512:1024])
                else:
                    trig = qkT[1][0:1, 0:1]
                    for p in range(NPAIR):
                        deferred_dma(trig, wo_sb[p][0:1, 0:1],
                                     wo_sb[p], wo_c[p])

            # ---- chain emitters ---------------------------------------
            def emit_qk_chain(p, tt):
                """qkT[p][:, tt*QT:...] = (w_qk[:, p-slice].T @ x.T) + bias."""
                ps = psA.tile([128, QT], F32, tag="pa", name=f"psqk{p}_{tt}")
                c0 = _wqk_col(p)
                for ch in range(NCH):
                    nc.tensor.matmul(
                        ps, wqk_sb[ch][:, c0:c0 + 128],
                        xT[ch][:, tt * QT:(tt + 1) * QT],
                        start=(ch == 0), stop=(ch == NCH - 1))
                nc.vector.tensor_scalar_add(
                    qkT[p][:, tt * QT:(tt + 1) * QT], ps, bqk_sb[:, p:p + 1])
                if (p, tt) == (0, 1):
                    emit_tier(3)
                elif (p, tt) == (1, 0):
                    emit_tier(4)

            def emit_v_chain(tt):
                """vext[tt] <- x[tt-tokens] @ w_v + bias, plus ones column."""
                ps = psA.tile([128, DC], F32, tag="pa", name=f"psv{tt}")
                for ch in range(NCH):
                    nc.tensor.matmul(
                        ps, xT[ch][:, tt * 128:(tt + 1) * 128], wv_sb[ch],
                        start=(ch == 0), stop=(ch == NCH - 1))
                src3 = ps.rearrange("p (h d) -> p h d", h=HPC)
                bv3 = bv_bc.rearrange("p (h d) -> p h d", h=HPC)
                nc.vector.tensor_add(vext[tt][:, :, 0:HD], src3, bv3)
                nc.vector.memset(vext[tt][:, :, HD:HD + 1], 1.0)

            def emit_d_chain(grp, tt, nt):
                """Partial out-projection for pair group grp (pairs 2g,2g+1)."""
                pairs = [2 * grp, 2 * grp + 1]
                dst = outA_d if grp == 0 else outB_d
                pso = psA.tile([128, QT], F32, name=f"pso{grp}_{tt}_{nt}",
                               tag="pa")
                for i, p in enumerate(pairs):
                    nc.tensor.matmul(
                        pso, outTn[p][:, tt * 128:(tt + 1) * 128],
                        wo_sb[p][:, nt * QT:(nt + 1) * QT],
                        start=(i == 0), stop=(i == len(pairs) - 1))
                ot = dcopy.tile([128, QT], F16, tag="oc")
                nc.vector.tensor_copy(ot, pso)
                nc.sync.dma_start(
                    dst.ap()[tt * 128:(tt + 1) * 128,
                             nt * QT:(nt + 1) * QT], ot)

            # ---- filler machinery -------------------------------------
            emitted = set()
            filler_q = collections.deque()
            state = {"quota": 0.0}

            def emit_unit(key):
                if key in emitted:
                    return
                emitted.add(key)
                if key[0] == "qk":
                    emit_qk_chain(key[1], key[2])
                elif key[0] == "v":
                    emit_v_chain(key[1])
                else:
                    emit_d_chain(key[1], key[2], key[3])

            def pop_fillers():
                while filler_q:
                    cost, key = filler_q[0]
                    if key in emitted:
                        filler_q.popleft()
                        continue
                    if state["quota"] < cost:
                        break
                    filler_q.popleft()
                    state["quota"] -= cost
                    emit_unit(key)

            # static queue: p0's remaining qk/v deps, then later pairs' qk
            for tt in range(1, NQT):
                filler_q.append((8, ("qk", 0, tt)))
                filler_q.append((8, ("qk", NPAIR, tt)))
                for kt in range(4 * tt, 4 * tt + 4):
                    filler_q.append((8, ("v", kt)))
            for p in range(1, NPAIR):
                for tt in range(NQT):
                    filler_q.append((8, ("qk", p, tt)))
                    filler_q.append((8, ("qk", NPAIR + p, tt)))

            # ---- attention emitters -----------------------------------
            oT_live = {}
            pAB_live = {}

            def emit_sexp(ev):
                p, qt, kt, first, last = ev
                qTp, kTp = qkT[p], qkT[NPAIR + p]
                if first:
                    oT_live[(p, qt)] = [
                        psO.tile([HD + 1, QT], F32,
                                 name=f"o{p}_{qt}_{h}", tag=f"o_{h}")
                        for h in range(2)]
                zlo, mlo, mhi = (0, 0, 0) if classes[kt, qt] == 1 \
                    else bounds[(kt, qt)]
                sAB = psS.tile([128, 2, QT], F32, tag="sAB",
                               name=f"s{p}_{qt}_{kt}")
                for h in range(2):
                    nc.tensor.matmul(
                        sAB[:, h, zlo:QT],
                        kTp[64 * h:64 * h + 64, kt * KT:(kt + 1) * KT],
                        qTp[64 * h:64 * h + 64, qt * QT + zlo:(qt + 1) * QT],
                        tile_position=(64 * h, 0))
                pAB = ppool.tile([128, 2, QT], F16, tag="pAB",
                                 name=f"p{p}_{qt}_{kt}")
                nc.scalar.activation(
                    pAB[:, :, zlo:QT], sAB[:, :, zlo:QT],
                    mybir.ActivationFunctionType.Exp, scale=SCALE)
                if mhi > mlo:
                    pap = mpat[pat_idx[(kt, qt)]][:, mlo:mhi]
                    bap = bass.AP(tensor=pap.tensor, offset=pap.offset,
                                  ap=[pap.ap[0], [0, 2]] + pap.ap[1:])
                    nc.vector.tensor_mul(
                        pAB[:, :, mlo:mhi], pAB[:, :, mlo:mhi], bap)
                if first and zlo:
                    # general-mask guard: first event must initialize the
                    # full oT width, so zero the dead columns and run av
                    # untrimmed (never hit for a causal mask: zlo==0).
                    nc.vector.memset(pAB[:, :, 0:zlo], 0.0)
                    zlo = 0
                pAB_live[(p, qt, kt)] = (pAB, zlo)

            def emit_av(ev):
                p, qt, kt, first, last = ev
                oT = oT_live[(p, qt)]
                pAB, zlo = pAB_live.pop((p, qt, kt))
                for h in range(2):
                    nc.tensor.matmul(
                        oT[h][:, zlo:QT], vext[kt][:, 2 * p + h, :],
                        pAB[:, h, zlo:QT], start=first, stop=last)
                if last:
                    for h in range(2):
                        den = spool.tile([1, QT], F32, tag="den",
                                         name=f"d{p}_{qt}_{h}")
                        nc.vector.tensor_copy(den, oT[h][HD:HD + 1, :])
                        rec = spool.tile([1, QT], F32, tag="rec",
                                         name=f"r{p}_{qt}_{h}")
                        nc.vector.reciprocal_approx_fast(out=rec, in_=den)
                        bc = spool.tile([HD, QT], F32, tag="bc",
                                        name=f"b{p}_{qt}_{h}")
                        nc.gpsimd.partition_broadcast(bc, rec[0:1, :])
                        nc.vector.tensor_mul(
                            outTn[p][64 * h:64 * h + 64,
                                     qt * QT:(qt + 1) * QT],
                            oT[h][0:HD, :], bc)
                    del oT_live[(p, qt)]
                    if p in (1, 3):
                        # pair-group rows for this qt are now final on both
                        # pairs; out-projection chains become emittable
                        for tt in range(4 * qt, 4 * qt + 4):
                            for nt in range(2):
                                filler_q.append((2, ("d", p // 2, tt, nt)))

            # ---- ramp: first qk chains, then v ------------------------
            with nc.named_scope("ramp"):
                emit_unit(("qk", 0, 0))
                emit_unit(("qk", NPAIR, 0))
                emit_tier(2)
                for kt in range(4):
                    emit_unit(("v", kt))

            # ---- main woven stream ------------------------------------
            all_events = []
            for p in range(NPAIR):
                for qt in range(NQT):
                    kts = [kt for kt in range(NKT) if classes[kt, qt] != 0]
                    for i, kt in enumerate(kts):
                        all_events.append(
                            (p, qt, kt, i == 0, i == len(kts) - 1))

            with nc.named_scope("attn"):
                for idx, ev in enumerate(all_events):
                    p, qt, kt, first, last = ev
                    emit_unit(("qk", p, qt))
                    emit_unit(("qk", NPAIR + p, kt // 4))
                    emit_unit(("v", kt))
                    emit_sexp(ev)
                    state["quota"] += RATE
                    j = idx - LAG
                    if j >= 0:
                        emit_av(all_events[j])
                    pop_fillers()
                for j in range(max(0, len(all_events) - LAG),
                               len(all_events)):
                    emit_av(all_events[j])

            with nc.named_scope("tail"):
                state["quota"] = float("inf")
                pop_fillers()

    nc.compile()
    return nc


def _prepare_inputs(x, mask, w_qkv, b_qkv, w_out):
    classes, patterns, pat_idx, bounds = _classify_mask(np.asarray(mask))
    in_maps = []
    for c in range(NCORES):
        b, g = c // 2, c % 2
        h0 = g * HPC
        xT = np.ascontiguousarray(x[b].T.astype(np.float16))
        wq = w_qkv[:, h0 * HD:h0 * HD + DC]
        wk = w_qkv[:, D + h0 * HD:D + h0 * HD + DC]
        wv = w_qkv[:, 2 * D + h0 * HD:2 * D + h0 * HD + DC]
        bq = b_qkv[h0 * HD:h0 * HD + DC]
        bk = b_qkv[D + h0 * HD:D + h0 * HD + DC]
        bv = b_qkv[2 * D + h0 * HD:2 * D + h0 * HD + DC]
        wo = w_out[h0 * HD:h0 * HD + DC, :]
        in_maps.append({
            "xT": xT,
            # column layout [q01 | k01 | q23 | k23] so the first half is
            # one contiguous ramp-critical DMA (see _wqk_col)
            "wqk": np.ascontiguousarray(
                np.concatenate([wq[:, 0:256], wk[:, 0:256],
                                wq[:, 256:512], wk[:, 256:512]],
                               axis=1).astype(np.float16)),
            "wv": np.ascontiguousarray(wv.astype(np.float16)),
            "wo": np.ascontiguousarray(wo.astype(np.float16)),
            "bqk": np.ascontiguousarray(
                np.concatenate([bq, bk]).reshape(2 * NPAIR, 128).T
                .astype(np.float32)),
            "bv": np.ascontiguousarray(bv[None, :].astype(np.float32)),
            "mpat": patterns.astype(np.float16),
        })
    return classes, patterns, pat_idx, bounds, in_maps


def run(x, mask, w_qkv, b_qkv, w_out, b_out, trace=False):
    classes, patterns, pat_idx, bounds, in_maps = _prepare_inputs(
        x, mask, w_qkv, b_qkv, w_out)
    key = (classes.tobytes(), patterns.tobytes())
    if key not in _cache:
        _cache[key] = _build(classes, pat_idx, bounds, patterns.shape[0])
    nc = _cache[key]
    res = bass_utils.run_bass_kernel_spmd(
        nc, in_maps, core_ids=list(range(NCORES)), trace=trace)
    out = np.empty((B, S, D), np.float32)
    bo = np.asarray(b_out, np.float32)
    for b in range(B):
        acc = np.zeros((S, D), np.float32)
        for c in (2 * b, 2 * b + 1):
            acc += res.results[c]["outA"].astype(np.float32)
            acc += res.results[c]["outB"].astype(np.float32)
        out[b] = acc + bo
    return out, res


def kernel(x, mask, w_qkv, b_qkv, w_out, b_out):
    out, _ = run(x, mask, w_qkv, b_qkv, w_out, b_out, trace=False)
    return out
